# revision 7
# baseline (speedup 1.0000x reference)
"""2-layer GAT on 8 Trainium2 NeuronCores (Bass/Tile, SPMD via axon PJRT).

Strategy (dst-sharded message passing, 3 launches, no collectives):
  A: per-core feature transform of its node shard: h~ = x @ (W1 R) and
     alpha_dst = x @ (W1 A1d). R is a per-head invertible rotation whose
     first column is a1_src, so alpha_src of a gathered row is just its
     strided column 16h -- no separate alpha table gather needed.
  B: layer-1 message passing. Edge slots laid out node-major per 128-node
     destination tile (slot j of node p = chunk j, partition p), so the
     PSUM-accumulating matmul uses a constant identity lhsT. Sources are
     gathered from two DRAM half-tables (int16 gather-index limit) as two
     per-tile slot streams. Per chunk: e = g[:,0:128:16] + a_d; Lrelu; Exp
     -> M[:,0:8] (denominator cols); M[:,8:136] = g * ex; one matmul
     accumulates denom+agg. Finalize: divide, un-rotate (PE transpose +
     matmul by R^-1), +b1, ReLU -> out1T, then fused layer-2 transform
     h2~ = relu(h1) @ [W2 | W2 a2s | W2 a2d] written to DRAM.
  C: layer-2 message passing over the same slot structure (row =
     [h2(40) | alpha2_src | pad] fp32, 256B), finalize with divide, +b2,
     log_softmax.
Host does only sharding glue: edge partitioning/sorting, half balancing,
permutations, table assembly between launches, constants.
"""
import sys
sys.path.insert(0, "/opt/trn_rl_repo")

import numpy as np
import jax

import concourse.bass as bass
import concourse.tile as tile
import concourse.mybir as mybir
from concourse import bacc
from concourse.bass2jax import _bass_exec_p, partition_id_tensor, install_neuronx_cc_hook
from jax.sharding import Mesh, PartitionSpec
from jax.experimental.shard_map import shard_map

F32 = mybir.dt.float32
I16 = mybir.dt.int16
AF = mybir.ActivationFunctionType
ALU = mybir.AluOpType

NEG_SLOPE = 0.2
DUMMY_ALPHA = -30000.0
P = 128


# ----------------------------------------------------------------------------
# configuration (sizes hardcoded for the graded problem; small configs used by
# the self-test harness pass explicit cfg)
# ----------------------------------------------------------------------------
class Cfg:
    def __init__(self, N, E, in_c=128, hid=16, heads=8, out_c=40, ncores=8):
        self.N, self.E = N, E
        self.in_c, self.hid, self.heads, self.out_c = in_c, hid, heads, out_c
        self.ncores = ncores
        self.npc = N // ncores                      # real nodes per core
        self.ntiles = -(-self.npc // P)             # dst tiles per core
        self.npad = self.ntiles * P                 # padded nodes per core
        # source table halves: node table slot range, dummy at local HALF
        tot = N
        self.half = -(-tot // 2)
        self.half = ((self.half + P - 1) // P) * P  # round half size up
        assert self.half + 1 <= 32767, "int16 gather index limit"
        self.c1 = heads * hid                       # layer-1 out channels (128)
        self.row2 = 64                              # layer-2 table row elems


CFG = Cfg(N=50000, E=800000)


# ----------------------------------------------------------------------------
# host-side math constants
# ----------------------------------------------------------------------------
def householder_rot(a):
    """R [k,k] with R[:,0] = a exactly, other columns orthonormal; plus R^-1."""
    k = a.shape[0]
    a = a.astype(np.float64)
    s = np.linalg.norm(a)
    if s < 1e-30:
        R = np.eye(k)
        R[0, 0] = 1.0
        return R, np.linalg.inv(R)
    u = a / s
    if u[0] > 1.0 - 1e-12:
        H = np.eye(k)
    else:
        v = u - np.eye(k)[:, 0]
        H = np.eye(k) - 2.0 * np.outer(v, v) / (v @ v)
    R = H @ np.diag([s] + [1.0] * (k - 1))
    Rinv = np.diag([1.0 / s] + [1.0] * (k - 1)) @ H
    return R, Rinv


def make_consts(cfg, W1, a1_src, a1_dst, W2, a2_src, a2_dst):
    """W1ext [in_c, c1+heads], Rinv_bd [c1, c1], W2ext [c1, out_c+2]."""
    H, D = cfg.heads, cfg.hid
    Rbd = np.zeros((cfg.c1, cfg.c1))
    Rinv = np.zeros((cfg.c1, cfg.c1))
    for h in range(H):
        R_h, Rinv_h = householder_rot(a1_src[h].astype(np.float64))
        Rbd[h * D:(h + 1) * D, h * D:(h + 1) * D] = R_h
        Rinv[h * D:(h + 1) * D, h * D:(h + 1) * D] = Rinv_h
    A1d = np.zeros((cfg.c1, H))
    for h in range(H):
        A1d[h * D:(h + 1) * D, h] = a1_dst[h].astype(np.float64)
    W1e = np.concatenate([W1.astype(np.float64) @ Rbd,
                          W1.astype(np.float64) @ A1d], axis=1)
    W2e = np.concatenate([W2.astype(np.float64),
                          W2.astype(np.float64) @ a2_src[0].astype(np.float64)[:, None],
                          W2.astype(np.float64) @ a2_dst[0].astype(np.float64)[:, None]],
                         axis=1)
    return (W1e.astype(np.float32), Rinv.astype(np.float32),
            W2e.astype(np.float32))


# ----------------------------------------------------------------------------
# host-side graph preprocessing
# ----------------------------------------------------------------------------
def balance_halves(cfg, src, dst):
    """Assign each node a half bit, balancing each dst's in-edges between
    halves (cuts per-tile max-degree padding). Greedy over sources."""
    N = cfg.N
    order = np.argsort(src, kind="stable")
    ssrc = src[order]
    sdst = dst[order]
    starts = np.searchsorted(ssrc, np.arange(N + 1))
    imb = np.zeros(N, dtype=np.int32)
    halfbit = np.zeros(N, dtype=np.int8)
    cap = cfg.half
    cnt = [0, 0]
    for s in range(N):
        lo, hi = starts[s], starts[s + 1]
        d = sdst[lo:hi]
        bias = int(imb[d].sum())
        h = 1 if bias > 0 else 0
        if cnt[h] >= cap:
            h = 1 - h
        halfbit[s] = h
        cnt[h] += 1
        np.add.at(imb, d, 1 - 2 * h)
    for _ in range(3):  # refinement sweeps
        for s in range(N):
            lo, hi = starts[s], starts[s + 1]
            d = sdst[lo:hi]
            h = int(halfbit[s])
            np.add.at(imb, d, -(1 - 2 * h))
            bias = int(imb[d].sum())
            hn = 1 if bias > 0 else 0
            if hn != h and cnt[hn] >= cap:
                hn = h
            if hn != h:
                cnt[h] -= 1
                cnt[hn] += 1
                halfbit[s] = hn
            np.add.at(imb, d, 1 - 2 * int(halfbit[s]))
    # local table slot within the half, by original id order
    sloc = np.zeros(N, dtype=np.int32)
    for h in (0, 1):
        m = halfbit == h
        sloc[m] = np.arange(int(m.sum()), dtype=np.int32)
        assert m.sum() <= cap
    return halfbit, sloc


def preprocess(cfg, edge_index):
    """Build all per-core edge-slot structures. Returns dict."""
    N, E, C = cfg.N, cfg.E, cfg.ncores
    loop = np.arange(N, dtype=np.int64)
    src = np.concatenate([edge_index[0], loop]).astype(np.int64)
    dst = np.concatenate([edge_index[1], loop]).astype(np.int64)
    halfbit, sloc = balance_halves(cfg, src, dst)

    # snake-deal sharding: global degree-desc order, node i -> core i%C,
    # rank i//C. gids[c][r] = global node id at (core c, rank r).
    degg = np.bincount(dst, minlength=N)
    gorder = np.argsort(-degg, kind="stable")
    node2core = np.empty(N, np.int32)
    node2rank = np.empty(N, np.int32)
    node2core[gorder] = (np.arange(N) % C).astype(np.int32)
    node2rank[gorder] = (np.arange(N) // C).astype(np.int32)
    gids = []
    for c in range(C):
        g = np.full(cfg.npad, -1, np.int64)
        sel = gorder[c::C]
        g[:len(sel)] = sel
        gids.append(g)
    deg_hc = np.zeros((C, cfg.npad, 2), dtype=np.int32)
    hbe = halfbit[src]
    for h in (0, 1):
        m = hbe == h
        np.add.at(deg_hc, (node2core[dst[m]], node2rank[dst[m]],
                           np.full(int(m.sum()), h)), 1)

    # common per-tile deltas across cores (same program on all cores)
    d0t = deg_hc[:, :, 0].reshape(C, cfg.ntiles, P).max(axis=(0, 2)).astype(np.int32)
    d1t = deg_hc[:, :, 1].reshape(C, cfg.ntiles, P).max(axis=(0, 2)).astype(np.int32)
    stot = int(128 * (d0t.sum() + d1t.sum()))
    stot16 = ((stot + 15) // 16) * 16

    # slot base position of each (tile, stream)
    bases = np.zeros((cfg.ntiles, 2), dtype=np.int64)
    pos = 0
    for t in range(cfg.ntiles):
        bases[t, 0] = pos
        pos += 128 * int(d0t[t])
        bases[t, 1] = pos
        pos += 128 * int(d1t[t])

    dummy = cfg.half  # dummy row local index in each half table
    idx_flats = []
    for c in range(C):
        m = node2core[dst] == c
        s_c = src[m]
        hb = halfbit[s_c].astype(np.int32)
        r = node2rank[dst[m]]
        t = r // P
        part = r % P
        key = (t.astype(np.int64) * 2 + hb) * cfg.npad + r
        order = np.argsort(key, kind="stable")
        ks = key[order]
        # occurrence j of each edge within its (node, half) group
        grp = (ks[1:] != ks[:-1]).cumsum()
        grp = np.concatenate([[0], grp])
        first = np.zeros(len(ks), dtype=np.int64)
        starts_ = np.flatnonzero(np.concatenate([[1], ks[1:] != ks[:-1]]))
        first[starts_] = 1
        gstart = np.repeat(np.arange(len(ks))[first.astype(bool)],
                           np.diff(np.concatenate([starts_, [len(ks)]])))
        j = np.arange(len(ks)) - gstart
        pos_ = (bases[t[order], hb[order]] + j * 128 + part[order])
        idx_flat = np.full(stot16, dummy, dtype=np.int16)
        idx_flat[pos_] = sloc[s_c[order]].astype(np.int16)
        idx_flats.append(idx_flat)

    # wrap to [128, stot16//16] with 8x replication
    idxws = []
    for c in range(C):
        w = np.zeros((P, stot16 // 16), dtype=np.int16)
        i = np.arange(stot16)
        w[i % 16, i // 16] = idx_flats[c]
        for r_ in range(1, 8):
            w[r_ * 16:(r_ + 1) * 16] = w[:16]
        idxws.append(w)

    return dict(src=src, dst=dst, halfbit=halfbit, sloc=sloc, gids=gids,
                d0t=d0t, d1t=d1t, bases=bases, stot16=stot16, idxws=idxws)


# ----------------------------------------------------------------------------
# SPMD runner (cached jit, modeled on bass2jax.run_bass_via_pjrt)
# ----------------------------------------------------------------------------
class SpmdRunner:
    def __init__(self, nc, n_cores, donate=True):
        install_neuronx_cc_hook()
        self.nc, self.n_cores = nc, n_cores
        pname = nc.partition_id_tensor.name if nc.partition_id_tensor else None
        in_names, out_names, out_avals, zero_outs = [], [], [], []
        for alloc in nc.m.functions[0].allocations:
            if not isinstance(alloc, mybir.MemoryLocationSet):
                continue
            name = alloc.memorylocations[0].name
            if alloc.kind == "ExternalInput":
                if name != pname:
                    in_names.append(name)
            elif alloc.kind == "ExternalOutput":
                out_names.append(name)
                shape = tuple(alloc.tensor_shape)
                dtype = mybir.dt.np(alloc.dtype)
                out_avals.append(jax.core.ShapedArray(shape, dtype))
                zero_outs.append(np.zeros(shape, dtype))
        self.n_params, self.in_names, self.out_names = len(in_names), in_names, out_names
        self.zero_outs = zero_outs
        all_in = in_names + out_names + ([pname] if pname else [])

        def _body(*args):
            ops = list(args)
            if pname is not None:
                ops.append(partition_id_tensor())
            return tuple(_bass_exec_p.bind(
                *ops, out_avals=tuple(out_avals), in_names=tuple(all_in),
                out_names=tuple(out_names), lowering_input_output_aliases=(),
                sim_require_finite=False, sim_require_nnan=False, nc=nc))

        dn = tuple(range(self.n_params, self.n_params + len(out_names))) \
            if donate else ()
        devices = jax.devices()[:n_cores]
        mesh = Mesh(np.asarray(devices), ("core",))
        ispec = (PartitionSpec("core"),) * (self.n_params + len(out_names))
        ospec = (PartitionSpec("core"),) * len(out_names)
        self.fn = jax.jit(shard_map(_body, mesh=mesh, in_specs=ispec,
                                    out_specs=ospec, check_rep=False),
                          donate_argnums=dn, keep_unused=True)

    def put_inputs(self, in_maps):
        concat = [np.concatenate([np.asarray(m[n]) for m in in_maps], axis=0)
                  for n in self.in_names]
        return [jax.device_put(x) for x in concat]

    def run(self, dev_inputs, retries=2):
        import time as _time
        for att in range(retries + 1):
            try:
                zeros = [np.concatenate([z] * self.n_cores, axis=0)
                         for z in self.zero_outs]
                outs = self.fn(*dev_inputs, *zeros)
                jax.block_until_ready(outs)
                return outs
            except Exception:
                if att == retries:
                    raise
                _time.sleep(60)

    def results(self, outs):
        res = [dict() for _ in range(self.n_cores)]
        for i, name in enumerate(self.out_names):
            for c, part in enumerate(np.split(np.asarray(outs[i]), self.n_cores)):
                res[c][name] = part
        return res


# ----------------------------------------------------------------------------
# launch A: h~ = x @ W1ext (per-core shard, pi-order)
# ----------------------------------------------------------------------------
def build_launchA(cfg, rep=1):
    nc = bacc.Bacc("TRN2", target_bir_lowering=False, debug=False,
                   num_devices=cfg.ncores)
    w = cfg.c1 + cfg.heads
    xT = nc.dram_tensor("xT", [cfg.in_c, cfg.npad], F32, kind="ExternalInput")
    W1e = nc.dram_tensor("W1e", [cfg.in_c, w], F32, kind="ExternalInput")
    hrows = nc.dram_tensor("hrows", [cfg.npad, cfg.c1], F32, kind="ExternalOutput")
    adrows = nc.dram_tensor("adrows", [cfg.npad, cfg.heads], F32, kind="ExternalOutput")
    with tile.TileContext(nc) as tc:
        with tc.tile_pool(name="fix", bufs=1) as fix, \
             tc.tile_pool(name="sb", bufs=4) as sb, \
             tc.tile_pool(name="ps", bufs=4, space="PSUM") as ps:
            wt = fix.tile([cfg.in_c, w], F32)
            nc.sync.dma_start(out=wt[:], in_=W1e[:, :])
            for _ in range(rep):
              for t in range(cfg.ntiles):
                  lhs = sb.tile([cfg.in_c, P], F32, tag="lhs")
                  nc.sync.dma_start(out=lhs[:], in_=xT[:, t * P:(t + 1) * P])
                  pt = ps.tile([P, w], F32, tag="ps")
                  nc.tensor.matmul(pt[:], lhsT=lhs[:], rhs=wt[:], start=True, stop=True)
                  ot = sb.tile([P, w], F32, tag="o")
                  nc.vector.tensor_copy(ot[:], pt[:])
                  nc.sync.dma_start(out=hrows[t * P:(t + 1) * P, :], in_=ot[:, :cfg.c1])
                  nc.sync.dma_start(out=adrows[t * P:(t + 1) * P, :], in_=ot[:, cfg.c1:])
    nc.compile()
    return nc


# ----------------------------------------------------------------------------
# launch B: layer-1 message passing + fused layer-2 feature transform
# ----------------------------------------------------------------------------
def build_launchB(cfg, d0t, d1t, stot16, rep=1):
    H = cfg.heads
    c1 = cfg.c1
    wm = H + c1          # M columns: [ex(H) | g*ex(c1)]
    nhalf = cfg.half + 1
    nc = bacc.Bacc("TRN2", target_bir_lowering=False, debug=False,
                   num_devices=cfg.ncores)
    tb0 = nc.dram_tensor("tb0", [nhalf, c1], F32, kind="ExternalInput")
    tb1 = nc.dram_tensor("tb1", [nhalf, c1], F32, kind="ExternalInput")
    idxs = nc.dram_tensor("idxs", [P, stot16 // 16], I16, kind="ExternalInput")
    adsw = nc.dram_tensor("adsw", [P, cfg.ntiles * H], F32, kind="ExternalInput")
    ident = nc.dram_tensor("ident", [P, P], F32, kind="ExternalInput")
    rinv = nc.dram_tensor("rinv", [c1, c1], F32, kind="ExternalInput")
    w2e = nc.dram_tensor("w2e", [c1, cfg.out_c + 2], F32, kind="ExternalInput")
    b1c = nc.dram_tensor("b1c", [c1, 1], F32, kind="ExternalInput")
    h2rows = nc.dram_tensor("h2rows", [cfg.npad, cfg.row2], F32, kind="ExternalOutput")

    dmax = int(max(d0t.max(), d1t.max()))
    with tile.TileContext(nc) as tc:
        with tc.tile_pool(name="fix", bufs=1) as fix, \
             tc.tile_pool(name="gp", bufs=3) as gp, \
             tc.tile_pool(name="mp", bufs=6) as mp, \
             tc.tile_pool(name="sm", bufs=8) as smp, \
             tc.tile_pool(name="fin", bufs=3) as fin, \
             tc.tile_pool(name="ps", bufs=2, space="PSUM") as ps, \
             tc.tile_pool(name="ps2", bufs=2, space="PSUM") as ps2, \
             tc.tile_pool(name="ps3", bufs=2, space="PSUM") as ps3, \
             tc.tile_pool(name="ps4", bufs=2, space="PSUM") as ps4:
            it = fix.tile([P, stot16 // 16], I16)
            nc.sync.dma_start(out=it[:], in_=idxs[:, :])
            ad = fix.tile([P, cfg.ntiles * H], F32)
            nc.sync.dma_start(out=ad[:], in_=adsw[:, :])
            idt = fix.tile([P, P], F32)
            nc.sync.dma_start(out=idt[:], in_=ident[:, :])
            riv = fix.tile([c1, c1], F32)
            nc.sync.dma_start(out=riv[:], in_=rinv[:, :])
            w2t = fix.tile([c1, cfg.out_c + 2], F32)
            nc.sync.dma_start(out=w2t[:], in_=w2e[:, :])
            b1t = fix.tile([c1, 1], F32)
            nc.sync.dma_start(out=b1t[:], in_=b1c[:, :])

            for _ in range(rep):
                pos = 0
                for t in range(cfg.ntiles):
                    deltas = [int(d0t[t]), int(d1t[t])]
                    nchunks = deltas[0] + deltas[1]
                    pt = ps.tile([P, wm], F32, tag="acc")
                    adt = ad[:, t * H:(t + 1) * H]
                    ci = 0
                    for sidx, tbl in ((0, tb0), (1, tb1)):
                        dlt = deltas[sidx]
                        if dlt == 0:
                            continue
                        gt = gp.tile([P, dmax * c1], F32, tag="g")
                        for o in range(0, dlt, 32):
                            seg = min(32, dlt - o)
                            nc.gpsimd.dma_gather(
                                out_ap=gt[:, o * c1:(o + seg) * c1]
                                    .rearrange("p (c e) -> p c e", e=c1),
                                in_ap=tbl[:, :],
                                idxs_ap=it[:, (pos + o * P) // 16:
                                           (pos + (o + seg) * P) // 16],
                                num_idxs=seg * P,
                                num_idxs_reg=seg * P,
                                elem_size=c1,
                                single_packet=False,
                            )
                        pos += dlt * P
                        for j in range(dlt):
                            g = gt[:, j * c1:(j + 1) * c1]
                            e = smp.tile([P, H], F32, tag="e")
                            nc.vector.tensor_tensor(
                                out=e[:], in0=g[:, 0:c1:cfg.hid], in1=adt,
                                op=ALU.add)
                            e2 = smp.tile([P, H], F32, tag="e2")
                            nc.scalar.activation(e2[:], e[:], AF.Lrelu,
                                                 alpha=NEG_SLOPE)
                            m = mp.tile([P, wm], F32, tag="m")
                            nc.scalar.activation(m[:, 0:H], e2[:], AF.Exp)
                            nc.vector.tensor_tensor(
                                out=m[:, H:wm].rearrange("p (h c) -> p h c",
                                                         c=cfg.hid),
                                in0=g.rearrange("p (h c) -> p h c", c=cfg.hid),
                                in1=m[:, 0:H].to_broadcast([P, H, cfg.hid]),
                                op=ALU.mult)
                            nc.tensor.matmul(pt[:], lhsT=idt[:], rhs=m[:],
                                             start=(ci == 0),
                                             stop=(ci == nchunks - 1))
                            ci += 1
                    # ---- finalize tile t ----
                    den = smp.tile([P, H], F32, tag="den")
                    nc.vector.tensor_scalar(out=den[:], in0=pt[:, 0:H],
                                            scalar1=1e-30, scalar2=None,
                                            op0=ALU.max)
                    rec = smp.tile([P, H], F32, tag="rec")
                    nc.vector.reciprocal(rec[:], den[:])
                    on = fin.tile([P, c1], F32, tag="on")
                    nc.vector.tensor_tensor(
                        out=on[:].rearrange("p (h c) -> p h c", c=cfg.hid),
                        in0=pt[:, H:wm].rearrange("p (h c) -> p h c", c=cfg.hid),
                        in1=rec[:].to_broadcast([P, H, cfg.hid]),
                        op=ALU.mult)
                    ptT = ps2.tile([P, P], F32, tag="pT")
                    nc.tensor.transpose(ptT[:], on[:], idt[:])
                    tT = fin.tile([c1, P], F32, tag="tT")
                    nc.vector.tensor_copy(tT[:], ptT[:])
                    p3 = ps3.tile([c1, P], F32, tag="p3")
                    nc.tensor.matmul(p3[:], lhsT=riv[:], rhs=tT[:],
                                     start=True, stop=True)
                    o1 = fin.tile([c1, P], F32, tag="o1")
                    nc.scalar.activation(o1[:], p3[:], AF.Relu, bias=b1t[:, 0:1])
                    p4 = ps4.tile([P, cfg.out_c + 2], F32, tag="p4")
                    nc.tensor.matmul(p4[:], lhsT=o1[:], rhs=w2t[:],
                                     start=True, stop=True)
                    h2 = fin.tile([P, cfg.row2], F32, tag="h2")
                    nc.vector.memset(h2[:], 0.0)
                    nc.vector.tensor_copy(h2[:, 0:cfg.out_c + 2], p4[:])
                    nc.sync.dma_start(out=h2rows[t * P:(t + 1) * P, :], in_=h2[:])
    nc.compile()
    return nc


# ----------------------------------------------------------------------------
# launch C: layer-2 message passing + log_softmax
# ----------------------------------------------------------------------------
def build_launchC(cfg, d0t, d1t, stot16, rep=1):
    oc = cfg.out_c
    wm = 1 + oc
    r2 = cfg.row2
    nhalf = cfg.half + 1
    nc = bacc.Bacc("TRN2", target_bir_lowering=False, debug=False,
                   num_devices=cfg.ncores)
    tb0 = nc.dram_tensor("tb0", [nhalf, r2], F32, kind="ExternalInput")
    tb1 = nc.dram_tensor("tb1", [nhalf, r2], F32, kind="ExternalInput")
    idxs = nc.dram_tensor("idxs", [P, stot16 // 16], I16, kind="ExternalInput")
    adsw = nc.dram_tensor("adsw", [P, cfg.ntiles], F32, kind="ExternalInput")
    ident = nc.dram_tensor("ident", [P, P], F32, kind="ExternalInput")
    b2c = nc.dram_tensor("b2c", [P, oc], F32, kind="ExternalInput")
    outr = nc.dram_tensor("outr", [cfg.npad, oc], F32, kind="ExternalOutput")

    dmax = int(max(d0t.max(), d1t.max()))
    with tile.TileContext(nc) as tc:
        with tc.tile_pool(name="fix", bufs=1) as fix, \
             tc.tile_pool(name="gp", bufs=3) as gp, \
             tc.tile_pool(name="mp", bufs=6) as mp, \
             tc.tile_pool(name="sm", bufs=8) as smp, \
             tc.tile_pool(name="fin", bufs=3) as fin, \
             tc.tile_pool(name="ps", bufs=2, space="PSUM") as ps:
            it = fix.tile([P, stot16 // 16], I16)
            nc.sync.dma_start(out=it[:], in_=idxs[:, :])
            ad = fix.tile([P, cfg.ntiles], F32)
            nc.sync.dma_start(out=ad[:], in_=adsw[:, :])
            idt = fix.tile([P, P], F32)
            nc.sync.dma_start(out=idt[:], in_=ident[:, :])
            b2t = fix.tile([P, oc], F32)
            nc.sync.dma_start(out=b2t[:], in_=b2c[:, :])

            for _ in range(rep):
                pos = 0
                for t in range(cfg.ntiles):
                    deltas = [int(d0t[t]), int(d1t[t])]
                    nchunks = deltas[0] + deltas[1]
                    pt = ps.tile([P, wm], F32, tag="acc")
                    adt = ad[:, t:t + 1]
                    ci = 0
                    for sidx, tbl in ((0, tb0), (1, tb1)):
                        dlt = deltas[sidx]
                        if dlt == 0:
                            continue
                        gt = gp.tile([P, dmax * r2], F32, tag="g")
                        for o in range(0, dlt, 32):
                            seg = min(32, dlt - o)
                            nc.gpsimd.dma_gather(
                                out_ap=gt[:, o * r2:(o + seg) * r2]
                                    .rearrange("p (c e) -> p c e", e=r2),
                                in_ap=tbl[:, :],
                                idxs_ap=it[:, (pos + o * P) // 16:
                                           (pos + (o + seg) * P) // 16],
                                num_idxs=seg * P,
                                num_idxs_reg=seg * P,
                                elem_size=r2,
                                single_packet=False,
                            )
                        pos += dlt * P
                        for j in range(dlt):
                            g = gt[:, j * r2:(j + 1) * r2]
                            e = smp.tile([P, 1], F32, tag="e")
                            nc.vector.tensor_tensor(
                                out=e[:], in0=g[:, oc:oc + 1], in1=adt,
                                op=ALU.add)
                            e2 = smp.tile([P, 1], F32, tag="e2")
                            nc.scalar.activation(e2[:], e[:], AF.Lrelu,
                                                 alpha=NEG_SLOPE)
                            m = mp.tile([P, wm], F32, tag="m")
                            nc.scalar.activation(m[:, 0:1], e2[:], AF.Exp)
                            nc.vector.tensor_scalar(
                                out=m[:, 1:wm], in0=g[:, 0:oc],
                                scalar1=m[:, 0:1], scalar2=None, op0=ALU.mult)
                            nc.tensor.matmul(pt[:], lhsT=idt[:], rhs=m[:],
                                             start=(ci == 0),
                                             stop=(ci == nchunks - 1))
                            ci += 1
                    # ---- finalize tile t: divide, +b2, log_softmax ----
                    den = smp.tile([P, 1], F32, tag="den")
                    nc.vector.tensor_scalar(out=den[:], in0=pt[:, 0:1],
                                            scalar1=1e-30, scalar2=None,
                                            op0=ALU.max)
                    rec = smp.tile([P, 1], F32, tag="rec")
                    nc.vector.reciprocal(rec[:], den[:])
                    o2 = fin.tile([P, oc], F32, tag="o2")
                    nc.vector.tensor_scalar(out=o2[:], in0=pt[:, 1:wm],
                                            scalar1=rec[:, 0:1], scalar2=None,
                                            op0=ALU.mult)
                    o2b = fin.tile([P, oc], F32, tag="o2b")
                    nc.vector.tensor_tensor(out=o2b[:], in0=o2[:], in1=b2t[:],
                                            op=ALU.add)
                    mx = smp.tile([P, 1], F32, tag="mx")
                    nc.vector.tensor_reduce(out=mx[:], in_=o2b[:],
                                            axis=mybir.AxisListType.X,
                                            op=ALU.max)
                    xs = fin.tile([P, oc], F32, tag="xs")
                    nc.vector.tensor_scalar(out=xs[:], in0=o2b[:],
                                            scalar1=mx[:, 0:1], scalar2=None,
                                            op0=ALU.subtract)
                    ex = fin.tile([P, oc], F32, tag="ex")
                    se = smp.tile([P, 1], F32, tag="se")
                    nc.scalar.activation(ex[:], xs[:], AF.Exp, accum_out=se[:])
                    ls = smp.tile([P, 1], F32, tag="ls")
                    nc.scalar.activation(ls[:], se[:], AF.Ln)
                    fo = fin.tile([P, oc], F32, tag="fo")
                    nc.vector.tensor_scalar(out=fo[:], in0=xs[:],
                                            scalar1=ls[:, 0:1], scalar2=None,
                                            op0=ALU.subtract)
                    nc.sync.dma_start(out=outr[t * P:(t + 1) * P, :], in_=fo[:])
    nc.compile()
    return nc


# ----------------------------------------------------------------------------
# full pipeline
# ----------------------------------------------------------------------------
def run_gat(cfg, inputs, timing=False, exec_fns=None):
    x = np.asarray(inputs["x"], dtype=np.float32)
    edge_index = np.asarray(inputs["edge_index"])
    W1e, Rinv, W2e = make_consts(
        cfg, np.asarray(inputs["W1"], np.float64),
        np.asarray(inputs["a1_src"], np.float64),
        np.asarray(inputs["a1_dst"], np.float64),
        np.asarray(inputs["W2"], np.float64),
        np.asarray(inputs["a2_src"], np.float64),
        np.asarray(inputs["a2_dst"], np.float64))
    b1 = np.asarray(inputs["b1"], np.float32)
    b2 = np.asarray(inputs["b2"], np.float32)
    pre = preprocess(cfg, edge_index)
    C = cfg.ncores

    def _default_exec(nc, maps):
        r = SpmdRunner(nc, C)
        return r.results(r.run(r.put_inputs(maps)))

    if exec_fns is None:
        exec_fns = {}

    # ---- launch A ----
    ncA = build_launchA(cfg)
    mapsA = []
    for c in range(C):
        g = pre["gids"][c]
        xp = np.zeros((cfg.npad, cfg.in_c), np.float32)
        valid = g >= 0
        xp[np.flatnonzero(valid)] = x[g[valid]]
        mapsA.append({"xT": np.ascontiguousarray(xp.T), "W1e": W1e})
    outsA = exec_fns.get("A", _default_exec)(ncA, mapsA)

    # assemble h~ table + alpha_d (pi-order per core)
    tblg = np.zeros((cfg.N, cfg.c1), np.float32)
    adsws = []
    for c in range(C):
        g = pre["gids"][c]
        valid = g >= 0
        tblg[g[valid]] = outsA[c]["hrows"][np.flatnonzero(valid)]
        adp = outsA[c]["adrows"]  # [npad, H] pi-order
        adsws.append(np.ascontiguousarray(
            adp.reshape(cfg.ntiles, P, cfg.heads).transpose(1, 0, 2)
               .reshape(P, cfg.ntiles * cfg.heads)))
    hb, sl = pre["halfbit"], pre["sloc"]
    nh = cfg.half + 1
    tb0 = np.zeros((nh, cfg.c1), np.float32)
    tb1 = np.zeros((nh, cfg.c1), np.float32)
    for h, tb in ((0, tb0), (1, tb1)):
        m = hb == h
        tb[sl[m]] = tblg[m]
        tb[cfg.half, 0:cfg.c1:cfg.hid] = DUMMY_ALPHA
    ident = np.eye(P, dtype=np.float32)

    # ---- launch B ----
    ncB = build_launchB(cfg, pre["d0t"], pre["d1t"], pre["stot16"])
    mapsB = [{"tb0": tb0, "tb1": tb1, "idxs": pre["idxws"][c],
              "adsw": adsws[c], "ident": ident, "rinv": Rinv, "w2e": W2e,
              "b1c": b1.reshape(-1, 1)} for c in range(C)]
    outsB = exec_fns.get("B", _default_exec)(ncB, mapsB)

    # assemble h2~ table + alpha2_d
    tbl2g = np.zeros((cfg.N, cfg.row2), np.float32)
    ad2sws = []
    for c in range(C):
        g = pre["gids"][c]
        valid = g >= 0
        h2r = outsB[c]["h2rows"]
        row = np.zeros((cfg.npad, cfg.row2), np.float32)
        row[:, 0:cfg.out_c + 1] = h2r[:, 0:cfg.out_c + 1]
        tbl2g[g[valid]] = row[np.flatnonzero(valid)]
        ad2 = h2r[:, cfg.out_c + 1]  # [npad] pi-order
        ad2sws.append(np.ascontiguousarray(
            ad2.reshape(cfg.ntiles, P).T))
    tb20 = np.zeros((nh, cfg.row2), np.float32)
    tb21 = np.zeros((nh, cfg.row2), np.float32)
    for h, tb in ((0, tb20), (1, tb21)):
        m = hb == h
        tb[sl[m]] = tbl2g[m]
        tb[cfg.half, cfg.out_c] = DUMMY_ALPHA

    # ---- launch C ----
    ncC = build_launchC(cfg, pre["d0t"], pre["d1t"], pre["stot16"])
    b2bc = np.tile(b2.reshape(1, -1), (P, 1)).astype(np.float32)
    mapsC = [{"tb0": tb20, "tb1": tb21, "idxs": pre["idxws"][c],
              "adsw": ad2sws[c], "ident": ident, "b2c": b2bc}
             for c in range(C)]
    outsC = exec_fns.get("C", _default_exec)(ncC, mapsC)

    out = np.zeros((cfg.N, cfg.out_c), np.float32)
    for c in range(C):
        g = pre["gids"][c]
        valid = g >= 0
        out[g[valid]] = outsC[c]["outr"][np.flatnonzero(valid)]
    return out


def kernel(**inputs) -> np.ndarray:
    return run_gat(CFG, inputs)

